# revision 1
# baseline (speedup 1.0000x reference)
"""Deformable PSROI pooling (group_size=1, num_classes=1) on 8 trn2 NeuronCores.

Strategy ("map sweep"):
  out[n, c, ph, pw] = sum_{y,x} KY[bin, y] * KX[bin, x] * data[b, c, y, x]
where KX/KY are per-bin bilinear "hat" weight profiles (sums over the 4x4
sample grid, with sample masks and 1/count folded in).  Each core holds a
slice of one batch's feature map in SBUF in [x(128 partitions), (y, c)]
layout; for each feature row y it issues one TensorE matmul
    psum[c, bins] += map_row[x, c].T @ W_y[x, bins]
accumulating bins in PSUM 512-column "generations" (bins sorted by first
active row).  W_y columns are dense per-bin x-profiles scaled by ky[bin, y],
built on the host and streamed per generation.

Sharding: bins are split by (batch, ylo-quantile) into 8 equal-count shards.
The compiled program is shared by all cores; all per-core variation lives in
the input tensors (map slice, W stream).  Cross-core schedule alignment is
per-generation ("virtual steps"), with the map supplied as per-generation row
segments so each core can anchor a generation at its own starting row.
"""
import sys
import time

import numpy as np

sys.path.insert(0, "/opt/trn_rl_repo")

SPATIAL_SCALE = np.float32(0.0625)
POOLED = 7
SAMPLES = 4
TRANS_STD = np.float32(0.1)
B, C, H, W = 2, 128, 128, 128
NCORES = 8
GEN_COLS = 512
DT_MODE = "f32"  # "f32" (exact, 4-pass PE) or "f32r" (1-pass, ~1.5e-4 rel err)

f32 = np.float32
YSENT = 10 ** 6  # sentinel ylo for bins with all-zero weights


# ----------------------------------------------------------------------------
# host planning
# ----------------------------------------------------------------------------

def _bin_params(rois, offset):
    """Exact float32 emulation of the reference coordinate math.

    Returns per-bin (N*49) arrays: batch, dense hat profiles kx/ky [nb, 128]
    (ky has 1/count folded in), y-support [ylo, yhi], zero-bin mask.
    """
    N = rois.shape[0]
    P, S = POOLED, SAMPLES
    rois = rois.astype(f32)
    offset = offset.astype(f32)

    batch_ind = rois[:, 0].astype(np.int32)
    roi_sw = np.round(rois[:, 1]) * SPATIAL_SCALE - f32(0.5)
    roi_sh = np.round(rois[:, 2]) * SPATIAL_SCALE - f32(0.5)
    roi_ew = np.round(rois[:, 3] + f32(1.0)) * SPATIAL_SCALE - f32(0.5)
    roi_eh = np.round(rois[:, 4] + f32(1.0)) * SPATIAL_SCALE - f32(0.5)
    roi_w = np.maximum(roi_ew - roi_sw, f32(0.1))
    roi_h = np.maximum(roi_eh - roi_sh, f32(0.1))
    bin_w = roi_w / f32(P)
    bin_h = roi_h / f32(P)
    sub_w = bin_w / f32(S)
    sub_h = bin_h / f32(S)

    pidx = np.arange(P, dtype=f32)
    trans_x = offset[:, 0] * TRANS_STD          # [N, 7(ph), 7(pw)]
    trans_y = offset[:, 1] * TRANS_STD
    pw = pidx[None, None, :]
    ph = pidx[None, :, None]
    wstart = pw * bin_w[:, None, None] + roi_sw[:, None, None] + trans_x * roi_w[:, None, None]
    hstart = ph * bin_h[:, None, None] + roi_sh[:, None, None] + trans_y * roi_h[:, None, None]

    sidx = np.arange(S, dtype=f32)
    w_s = wstart[..., None] + sidx * sub_w[:, None, None, None]     # [N,7,7,4]
    h_s = hstart[..., None] + sidx * sub_h[:, None, None, None]
    mask_w = (w_s >= f32(-0.5)) & (w_s <= f32(W) - f32(0.5))
    mask_h = (h_s >= f32(-0.5)) & (h_s <= f32(H) - f32(0.5))
    wc = np.clip(w_s, f32(0.0), f32(W - 1))
    hc = np.clip(h_s, f32(0.0), f32(H - 1))

    cnt = (mask_h.sum(-1) * mask_w.sum(-1)).astype(f32)             # [N,7,7]
    inv = np.where(cnt > 0, f32(1.0) / np.maximum(cnt, f32(1.0)), f32(0.0))

    nb = N * P * P
    wc = wc.reshape(nb, S)
    hc = hc.reshape(nb, S)
    mask_w = mask_w.reshape(nb, S)
    mask_h = mask_h.reshape(nb, S)
    inv = inv.reshape(nb)

    xg = np.arange(W, dtype=np.float64)
    kx = np.zeros((nb, W), np.float64)
    ky = np.zeros((nb, H), np.float64)
    for s in range(S):
        kx += mask_w[:, s, None] * np.maximum(0.0, 1.0 - np.abs(wc[:, s, None].astype(np.float64) - xg))
        ky += mask_h[:, s, None] * np.maximum(0.0, 1.0 - np.abs(hc[:, s, None].astype(np.float64) - xg))
    ky *= inv[:, None]
    kx = kx.astype(f32)
    ky = ky.astype(f32)

    ky_nz = ky != 0
    has_y = ky_nz.any(axis=1)
    ylo = np.where(has_y, ky_nz.argmax(axis=1), YSENT).astype(np.int64)
    yhi = np.where(has_y, H - 1 - ky_nz[:, ::-1].argmax(axis=1), -YSENT).astype(np.int64)

    batch = np.repeat(batch_ind, P * P)
    return batch, kx, ky, ylo, yhi, has_y


def _plan(rois, offset):
    batch, kx, ky, ylo, yhi, has_y = _bin_params(rois, offset)

    # shard bins: (batch, ylo-quantile) -> 8 shards with equal-ish counts
    shards = []
    for b in range(B):
        ids = np.where(batch == b)[0]
        # secondary yhi sort shrinks the retirement-prefix blocking window
        ids = ids[np.lexsort((yhi[ids], ylo[ids]))]
        q = NCORES // B
        shards.extend(ids[int(len(ids) * i / q):int(len(ids) * (i + 1) / q)]
                      for i in range(q))
    assert len(shards) == NCORES

    max_count = max(len(s) for s in shards)
    ngens = max(1, -(-max_count // GEN_COLS))
    nslots = ngens * GEN_COLS

    # per-shard absolute row window
    row_start = np.zeros(NCORES, np.int64)
    extents = []
    for ids in shards:
        real = ids[ylo[ids] < YSENT]
        if len(real):
            extents.append((int(ylo[real].min()), int(yhi[real].max())))
        else:
            extents.append((0, 0))
    rstar = max(b_ - a_ + 1 for a_, b_ in extents)
    rstar = min(H, -(-rstar // 8) * 8)
    for ci, (a_, b_) in enumerate(extents):
        row_start[ci] = min(a_, H - rstar)

    # per (core, gen): local anchor t0 and per-step lo/hi profiles
    t0 = np.zeros((NCORES, ngens), np.int64)         # anchor row (local)
    span = np.zeros((NCORES, ngens), np.int64)       # steps used by this core
    profiles = {}                                    # (ci, g) -> (los, his) arrays
    for ci, ids in enumerate(shards):
        rs = int(row_start[ci])
        for g in range(ngens):
            gids = ids[g * GEN_COLS:(g + 1) * GEN_COLS]
            real_mask = ylo[gids] < YSENT
            real = gids[real_mask]
            if len(real) == 0:
                continue
            yl_r = ylo[real] - rs
            yh_r = yhi[real] - rs
            a_, b_ = int(yl_r.min()), int(yh_r.max())
            t0[ci, g] = a_
            span[ci, g] = b_ - a_ + 1
            # profiles over the gen's slot list (real bins are a prefix of the
            # slot list because sentinels sort last)
            yl = ylo[gids] - rs
            yh = yhi[gids] - rs
            los, his = [], []
            for s in range(b_ - a_ + 1):
                t = a_ + s
                his.append(int(np.count_nonzero(yl <= t)))
                live = np.where(yh >= t)[0]
                los.append(int(live[0]) if len(live) else his[-1])
            profiles[(ci, g)] = (los, his)

    # shared schedule: per gen, steps s in [0, S_g); envelope widths
    sched = []          # (g, s, col_lo, col_hi, first, last)
    seg_rows = []       # steps per gen (map segment sizes)
    for g in range(ngens):
        S_g = int(span[:, g].max()) if span[:, g].max() > 0 else 0
        seg_rows.append(S_g)
        if S_g == 0:
            continue
        al = 8 if DT_MODE == "f32r" else 1   # fp32r ISA needs 8-aligned psum cols
        rows = []
        for s in range(S_g):
            cl, ch = YSENT, 0
            for ci in range(NCORES):
                if (ci, g) not in profiles:
                    continue
                los, his = profiles[(ci, g)]
                if s < len(los) and his[s] > los[s]:
                    cl = min(cl, los[s])
                    ch = max(ch, his[s])
            if ch > cl:
                rows.append((s, cl & ~(al - 1), min(GEN_COLS, -(-ch // al) * al)))
        for i, (s, cl, ch) in enumerate(rows):
            sched.append((g, s, cl, ch, i == 0, i == len(rows) - 1))
    wcols = sum(ch - cl for (_, _, cl, ch, _, _) in sched)
    srows = sum(seg_rows)

    # map segment offsets (rows) and W chunk offsets (cols) per gen
    seg_off = np.concatenate([[0], np.cumsum(seg_rows)]).astype(np.int64)
    gen_wc = {}
    off = 0
    for (g, s, cl, ch, first, last) in sched:
        if first:
            gen_wc[g] = off
        off += ch - cl

    meta = dict(rstar=int(rstar), ngens=int(ngens), nslots=int(nslots),
                sched=tuple(sched), wcols=int(wcols), srows=int(srows),
                seg_rows=tuple(seg_rows), seg_off=tuple(int(x) for x in seg_off),
                gen_wc=tuple(sorted(gen_wc.items())))
    return dict(meta=meta, shards=shards, row_start=row_start, t0=t0,
                kx=kx, ky=ky, ylo=ylo, has_y=has_y)


def _build_inputs(plan, data):
    meta = plan["meta"]
    sched, srows, wcols = meta["sched"], meta["srows"], meta["wcols"]
    seg_off = meta["seg_off"]
    kx, ky = plan["kx"], plan["ky"]
    t0 = plan["t0"]
    data_perm = np.ascontiguousarray(data.transpose(0, 3, 2, 1))  # [B, W(x), H(y), C]

    in_maps = []
    for ci in range(NCORES):
        ids = plan["shards"][ci]
        b = ci // (NCORES // B)
        rs = int(plan["row_start"][ci])
        # segmented map: for gen g, S_g rows starting at rs + t0[ci, g]
        mp = np.zeros((128, srows, C), f32)
        for g, S_g in enumerate(meta["seg_rows"]):
            if S_g == 0:
                continue
            y0 = rs + int(t0[ci, g])
            y1 = min(H, y0 + S_g)
            mp[:, seg_off[g]:seg_off[g] + (y1 - y0), :] = data_perm[b, :, y0:y1, :]
        wbuf = np.zeros((128, wcols), f32)
        wc_off = 0
        for (g, s, cl, ch, first, last) in sched:
            width = ch - cl
            y = rs + int(t0[ci, g]) + s
            if y < H:
                gids = ids[g * GEN_COLS + cl:g * GEN_COLS + ch]
                if len(gids):
                    vals = kx[gids] * ky[gids, y][:, None]      # [ncols_real, 128]
                    wbuf[:, wc_off:wc_off + len(gids)] = vals.T
            wc_off += width
        in_maps.append({"mp": np.ascontiguousarray(mp.reshape(128, srows * C)),
                        "w": wbuf})
    return in_maps


# ----------------------------------------------------------------------------
# device program
# ----------------------------------------------------------------------------

def _split_drains(nc, mybir, bass_rust):
    for f_ in nc.m.functions:
        for blk in f_.blocks:
            newlist = []
            for ins in blk.instructions:
                wts = list(ins.sync_info.on_wait) if ins.sync_info else []
                if len(wts) > 1 and type(ins).__name__ == "InstDrain":
                    for j, wx in enumerate(wts[1:]):
                        nop = mybir.InstNoOp(name=f"splitw_{id(ins)}_{j}", ins=[], outs=[])
                        nop.engine = ins.engine
                        nop.sync_info = bass_rust.SyncInfo(on_wait=[wx], on_update=[])
                        newlist.append(nop)
                    ins.sync_info.on_wait = wts[:1]
                newlist.append(ins)
            blk.instructions = newlist


def _build_program(meta, rep=1):
    import concourse.bacc as bacc
    import concourse.mybir as mybir
    import bass_rust
    from concourse.tile import TileContext

    ngens, nslots = meta["ngens"], meta["nslots"]
    sched, wcols, srows = meta["sched"], meta["wcols"], meta["srows"]
    seg_off = meta["seg_off"]
    gen_wc = dict(meta["gen_wc"])
    dt = mybir.dt.float32r if DT_MODE == "f32r" else mybir.dt.float32

    # per-gen W chunk extents
    gen_wend = {}
    off = 0
    for (g, s, cl, ch, first, last) in sched:
        off += ch - cl
        gen_wend[g] = off

    nc = bacc.Bacc()
    mp = nc.declare_dram_parameter("mp", [128, srows * C], dt, isOutput=False)
    w = nc.declare_dram_parameter("w", [128, max(wcols, 8)], dt, isOutput=False)
    o = nc.declare_dram_parameter("o", [128, nslots], mybir.dt.float32, isOutput=True)

    with TileContext(nc) as tc:
        with (
            tc.tile_pool(name="const", bufs=1) as constp,
            tc.tile_pool(name="mapp", bufs=2) as mpool,
            tc.tile_pool(name="wp", bufs=3) as wpool,
            tc.tile_pool(name="ps", bufs=2, space="PSUM") as pspool,
        ):
            stage = constp.tile([128, nslots], mybir.dt.float32)
            wmax = max((gen_wend[g] - gen_wc[g] for g in gen_wc), default=8)
            for _rep in range(rep):
                map_t = mpool.tile([128, srows * C], dt, tag="map")
                nload = min(16, srows)
                bounds = [int(srows * i / nload) for i in range(nload + 1)]
                for i in range(nload):
                    r0, r1 = bounds[i], bounds[i + 1]
                    if r1 > r0:
                        nc.sync.dma_start(out=map_t[:, r0 * C:r1 * C], in_=mp[:, r0 * C:r1 * C])
                ps = None
                w_t = None
                cur_g = -1
                wc_off = 0
                for (g, s, cl, ch, first, last) in sched:
                    width = ch - cl
                    if g != cur_g:
                        ps = pspool.tile([128, GEN_COLS], mybir.dt.float32, tag="ps")
                        w_t = wpool.tile([128, wmax], dt, tag="wt")
                        # W stream on the ACT HWDGE ring so it doesn't queue
                        # behind map-segment loads on the SP ring
                        nc.scalar.dma_start(out=w_t[:, :gen_wend[g] - gen_wc[g]],
                                            in_=w[:, gen_wc[g]:gen_wend[g]])
                        cur_g = g
                        wc_off = 0
                    row = map_t[:, (seg_off[g] + s) * C:(seg_off[g] + s + 1) * C]
                    nc.tensor.matmul(ps[:, cl:ch], row, w_t[:, wc_off:wc_off + width],
                                     start=first, stop=last)
                    wc_off += width
                    if last:
                        sl = slice(g * GEN_COLS, (g + 1) * GEN_COLS)
                        nc.vector.tensor_copy(stage[:, sl], ps[:])
                        # drain each generation's output immediately so the
                        # store overlaps later generations' compute
                        nc.sync.dma_start(out=o[:, sl], in_=stage[:, sl])

    _split_drains(nc, mybir, bass_rust)
    nc.finalize()
    return nc


_prog_cache = {}


def _get_program(meta, rep=1):
    key = (meta["sched"], meta["srows"], meta["nslots"], rep, DT_MODE)
    if key not in _prog_cache:
        _prog_cache[key] = _build_program(meta, rep=rep)
    return _prog_cache[key]


def _run(nc, in_maps):
    from concourse.bass_utils import run_bass_kernel_spmd
    last_err = None
    for _attempt in range(3):
        try:
            res = run_bass_kernel_spmd(nc, in_maps, list(range(NCORES)))
            return res.results
        except Exception as e:  # transient device wedge -> retry
            last_err = e
            time.sleep(2.0)
    raise last_err


# ----------------------------------------------------------------------------
# public entry
# ----------------------------------------------------------------------------

def kernel(data, rois, offset):
    data = np.asarray(data, f32)
    rois = np.asarray(rois, f32)
    offset = np.asarray(offset, f32)
    N = rois.shape[0]

    plan = _plan(rois, offset)
    if len(plan["meta"]["sched"]) == 0:   # every bin fully masked
        return np.zeros((N, C, POOLED, POOLED), f32)
    in_maps = _build_inputs(plan, data)
    nc = _get_program(plan["meta"])
    results = _run(nc, in_maps)

    flat = np.zeros((N * POOLED * POOLED, C), f32)   # [bin, c]
    for ci in range(NCORES):
        ids = plan["shards"][ci]
        if len(ids) == 0:
            continue
        sb = results[ci]["o"]  # [128, nslots]
        flat[ids] = sb[:, :len(ids)].T
    flat[~plan["has_y"]] = 0.0   # degenerate bins never touched on device
    out = flat.reshape(N, POOLED, POOLED, C).transpose(0, 3, 1, 2)
    return np.ascontiguousarray(out)



# revision 25
# speedup vs baseline: 4.2977x; 4.2977x over previous
"""Deformable PSROI pooling (group_size=1, num_classes=1) on 8 trn2 NeuronCores.

Strategy ("block sweep", v2):
  out[n, c, ph, pw] = sum_{y,x} KY[bin, y] * KX[bin, x] * data[b, c, y, x]
with KX/KY per-bin bilinear hat profiles (masks and 1/count folded in).
Support is tiny (<=5 rows x <=5 cols), so the contraction is blocked into
4-row x 32-col map blocks packed into the PE partition dim:
  map4[p = (y%4)*32 + (x%32), (yblk a, xblk b, c)]
One K=128 matmul per (gen, block) contracts a whole 4x32 block for all bins
of that block:   psum[c, cols] += map4[:, a, b, :].T @ W[:, cols]
where W[p, col] = KX[bin, 32b + p%32] * KY[bin, 4a + p//32]  (host-built).

Bins are grouped into "gens" by 4-row local ylo windows (one PSUM bank of
<=512 columns each). A bin with ylo in window a0 spills at most into block
a0+1 (support <= 5 rows); spill contributions use nested K=32 matmuls at
partition bands 32d (legal tile positions) so spill W columns carry only
32 rows instead of 128.

Sharding: RoI bins are split by (batch, ylo-quantile) into 8 shards; the
compiled program is shared, per-core variation lives in tensor contents.
Columns are per-core gen-local (cell-concat order); matmul column ranges are
cross-core envelopes; zero W entries make foreign columns harmless.
All streams are bf16 (tolerance 2e-2; bf16 error ~1e-3).
"""
import sys
import time

import numpy as np

sys.path.insert(0, "/opt/trn_rl_repo")

SPATIAL_SCALE = np.float32(0.0625)
POOLED = 7
SAMPLES = 4
TRANS_STD = np.float32(0.1)
B, C, H, W = 2, 128, 128, 128
NCORES = 8
GEN_COLS = 512
YWIN = 4
XWIN = 32
NXB = W // XWIN

f32 = np.float32
YSENT = 10 ** 6


def _bf16(a):
    import ml_dtypes
    return a.astype(ml_dtypes.bfloat16)


# ----------------------------------------------------------------------------
# host planning
# ----------------------------------------------------------------------------

def _bin_params(rois, offset):
    """Exact float32 emulation of the reference coordinate math.

    Returns per-bin (N*49) arrays: batch, dense hat profiles kx/ky [nb, 128]
    (ky has 1/count folded in), y-support [ylo, yhi], active mask.
    """
    N = rois.shape[0]
    P, S = POOLED, SAMPLES
    rois = rois.astype(f32)
    offset = offset.astype(f32)

    batch_ind = rois[:, 0].astype(np.int32)
    roi_sw = np.round(rois[:, 1]) * SPATIAL_SCALE - f32(0.5)
    roi_sh = np.round(rois[:, 2]) * SPATIAL_SCALE - f32(0.5)
    roi_ew = np.round(rois[:, 3] + f32(1.0)) * SPATIAL_SCALE - f32(0.5)
    roi_eh = np.round(rois[:, 4] + f32(1.0)) * SPATIAL_SCALE - f32(0.5)
    roi_w = np.maximum(roi_ew - roi_sw, f32(0.1))
    roi_h = np.maximum(roi_eh - roi_sh, f32(0.1))
    bin_w = roi_w / f32(P)
    bin_h = roi_h / f32(P)
    sub_w = bin_w / f32(S)
    sub_h = bin_h / f32(S)

    pidx = np.arange(P, dtype=f32)
    trans_x = offset[:, 0] * TRANS_STD
    trans_y = offset[:, 1] * TRANS_STD
    pw = pidx[None, None, :]
    ph = pidx[None, :, None]
    wstart = pw * bin_w[:, None, None] + roi_sw[:, None, None] + trans_x * roi_w[:, None, None]
    hstart = ph * bin_h[:, None, None] + roi_sh[:, None, None] + trans_y * roi_h[:, None, None]

    sidx = np.arange(S, dtype=f32)
    w_s = wstart[..., None] + sidx * sub_w[:, None, None, None]
    h_s = hstart[..., None] + sidx * sub_h[:, None, None, None]
    mask_w = (w_s >= f32(-0.5)) & (w_s <= f32(W) - f32(0.5))
    mask_h = (h_s >= f32(-0.5)) & (h_s <= f32(H) - f32(0.5))
    wc = np.clip(w_s, f32(0.0), f32(W - 1))
    hc = np.clip(h_s, f32(0.0), f32(H - 1))

    cnt = (mask_h.sum(-1) * mask_w.sum(-1)).astype(f32)
    inv = np.where(cnt > 0, f32(1.0) / np.maximum(cnt, f32(1.0)), f32(0.0))

    nb = N * P * P
    wc = wc.reshape(nb, S)
    hc = hc.reshape(nb, S)
    mask_w = mask_w.reshape(nb, S)
    mask_h = mask_h.reshape(nb, S)
    inv = inv.reshape(nb)

    xg = np.arange(W, dtype=np.float64)
    kx = np.zeros((nb, W), np.float64)
    ky = np.zeros((nb, H), np.float64)
    for s in range(S):
        kx += mask_w[:, s, None] * np.maximum(0.0, 1.0 - np.abs(wc[:, s, None].astype(np.float64) - xg))
        ky += mask_h[:, s, None] * np.maximum(0.0, 1.0 - np.abs(hc[:, s, None].astype(np.float64) - xg))
    ky *= inv[:, None]
    kx = kx.astype(f32)
    ky = ky.astype(f32)

    ky_nz = ky != 0
    kx_nz = kx != 0
    act = ky_nz.any(axis=1) & kx_nz.any(axis=1)
    ylo = np.where(act, ky_nz.argmax(axis=1), YSENT).astype(np.int64)
    yhi = np.where(act, H - 1 - ky_nz[:, ::-1].argmax(axis=1), -YSENT).astype(np.int64)
    xlo = np.where(act, kx_nz.argmax(axis=1), 0).astype(np.int64)
    xhi = np.where(act, W - 1 - kx_nz[:, ::-1].argmax(axis=1), 0).astype(np.int64)

    batch = np.repeat(batch_ind, P * P)
    return batch, kx, ky, ylo, yhi, xlo, xhi, act


def _plan(rois, offset):
    batch, kx, ky, ylo, yhi, xlo, xhi, act = _bin_params(rois, offset)

    # cell = home xblk; straddlers sort as a suffix inside their home cell
    home = np.clip(xlo // XWIN, 0, NXB - 1)
    strad = (xhi // XWIN) > home
    cell = home
    ncells = NXB

    # shard per batch into 4 ylo-quantile shards
    shards = []
    for b in range(B):
        ids = np.where((batch == b) & act)[0]
        ids = ids[np.lexsort((yhi[ids], ylo[ids]))]
        q = NCORES // B
        shards.extend(ids[int(len(ids) * i / q):int(len(ids) * (i + 1) / q)]
                      for i in range(q))
    assert len(shards) == NCORES

    # per-core 4-aligned row start
    row_start = np.zeros(NCORES, np.int64)
    for ci, ids in enumerate(shards):
        row_start[ci] = (int(ylo[ids].min()) // YWIN) * YWIN if len(ids) else 0
    nyb = 0
    for ci, ids in enumerate(shards):
        if len(ids):
            nyb = max(nyb, (int(yhi[ids].max()) - int(row_start[ci])) // YWIN + 1)
    nyb += 1  # room for the spill block of the last window
    ngens_max = nyb  # window index range

    # per (core, gen): bins with local ylo window == g, ordered by
    # (cell, straddle-flag, -yhi) -- straddlers are a sub-capped segment per
    # cell; within each segment spillers (yhi >= window end) are an exact
    # PREFIX (descending yhi), so spill envelopes anchor at the segment base
    core_gen = {}
    for ci, ids in enumerate(shards):
        g_of = (ylo[ids] - row_start[ci]) // YWIN
        for g in range(ngens_max):
            sub = ids[g_of == g]
            sub = sub[np.lexsort((-yhi[sub], strad[sub], cell[sub]))]
            core_gen[(ci, g)] = sub

    spans_ok = True
    for ids in shards:
        if len(ids):
            spans_ok &= bool((yhi[ids] - ylo[ids]).max() <= 2 * YWIN - 1)
            spans_ok &= bool((xhi[ids] - xlo[ids]).max() <= XWIN - 1)
    assert spans_ok, "bin support exceeds block-spill capacity"

    # units = (window g, cell e) groups; greedily packed into <=512-col banks.
    # mms entry: (a, b, cl, ch, wc); W values are support-driven (auto-zero
    # outside support), so one fill rule covers main/straddle-right/spill.
    units = []
    for g in range(ngens_max):
        if max(len(core_gen[(ci, g)]) for ci in range(NCORES)) == 0:
            continue
        for e in range(ncells):
            lp, ls = [], []
            for ci in range(NCORES):
                sub = core_gen[(ci, g)]
                mine = sub[cell[sub] == e]
                lp.append(mine[~strad[mine]])
                ls.append(mine[strad[mine]])
            capp = max(len(l_) for l_ in lp)
            caps_ = max(len(l_) for l_ in ls)
            if capp + caps_:
                units.append((g, e, capp, caps_, lp, ls))

    gens = []
    colbin = {}         # (ci, bank_idx) -> bin id per column (-1 hole)
    out_off = 0
    wm_cur = 0
    u0 = 0
    while u0 < len(units):
        u1 = u0 + 1
        cw = units[u0][2] + units[u0][3]
        while u1 < len(units) and cw + units[u1][2] + units[u1][3] <= GEN_COLS:
            cw += units[u1][2] + units[u1][3]
            u1 += 1
        capw = cw
        wm_lo = wm_cur
        mms = []
        for ci in range(NCORES):
            cb = -np.ones(capw, np.int64)
            b_ = 0
            for (g, e, capp, caps_, lp, ls) in units[u0:u1]:
                cb[b_:b_ + len(lp[ci])] = lp[ci]
                cb[b_ + capp:b_ + capp + len(ls[ci])] = ls[ci]
                b_ += capp + caps_
            colbin[(ci, len(gens))] = cb
        base = 0
        for (g, e, capp, caps_, lp, ls) in units[u0:u1]:
            T = YWIN * (g + 1)
            # spill prefix lengths (descending-yhi sort => exact prefix)
            nsp, nss = 0, 0
            for ci in range(NCORES):
                if len(lp[ci]):
                    nsp = max(nsp, int(np.count_nonzero(
                        yhi[lp[ci]] - row_start[ci] >= T)))
                if len(ls[ci]):
                    nss = max(nss, int(np.count_nonzero(
                        yhi[ls[ci]] - row_start[ci] >= T)))
            # main: block (g, e) over the whole unit range
            mms.append(dict(a=g, b=e, cl=base, ch=base + capp + caps_,
                            wc=wm_cur))
            wm_cur += capp + caps_
            # straddle-right: block (g, e+1) over the straddler sub-segment
            if caps_ and e + 1 < NXB:
                mms.append(dict(a=g, b=e + 1, cl=base + capp,
                                ch=base + capp + caps_, wc=wm_cur))
                wm_cur += caps_
            # spill: block (g+1, e) over the spill prefix(es)
            if nsp:
                mms.append(dict(a=g + 1, b=e, cl=base, ch=base + nsp,
                                wc=wm_cur))
                wm_cur += nsp
            if nss:
                mms.append(dict(a=g + 1, b=e, cl=base + capp,
                                ch=base + capp + nss, wc=wm_cur))
                wm_cur += nss
                if e + 1 < NXB:
                    mms.append(dict(a=g + 1, b=e + 1, cl=base + capp,
                                    ch=base + capp + nss, wc=wm_cur))
                    wm_cur += nss
            base += capp + caps_
        gens.append(dict(g=units[u0][0], capw=capw, out_off=out_off, mms=mms,
                         wm_lo=wm_lo, wm_hi=wm_cur))
        out_off += -(-capw // 8) * 8  # pad to 8 cols (16B bf16 alignment)
        u0 = u1
    nslots = out_off
    WCm = max(wm_cur, 8)

    # wm chunk boundaries (per ~2 gens) for streaming
    wm_chunks = []
    for i in range(0, len(gens), 2):
        j = min(i + 2, len(gens))
        wm_chunks.append((i, j, gens[i]["wm_lo"], gens[j - 1]["wm_hi"]))

    # hashable meta for the device program
    meta_gens = []
    for gd in gens:
        mt = tuple((m["a"], m["b"], m["cl"], m["ch"], m["wc"])
                   for m in gd["mms"])
        meta_gens.append((gd["g"], gd["capw"], gd["out_off"], mt))
    meta = dict(nyb=int(nyb), nslots=int(nslots), WCm=int(WCm),
                gens=tuple(meta_gens),
                wm_chunks=tuple(wm_chunks))
    return dict(meta=meta, gens=gens, row_start=row_start, colbin=colbin,
                kx=kx, ky=ky, act=act, batch=batch)


def _build_inputs(plan, data):
    meta = plan["meta"]
    nyb, WCm = meta["nyb"], meta["WCm"]
    kx, ky = plan["kx"], plan["ky"]
    gens, colbin, row_start = plan["gens"], plan["colbin"], plan["row_start"]

    in_maps = []
    for ci in range(NCORES):
        b = ci // (NCORES // B)
        rs = int(row_start[ci])
        # map4[p=(dy*32+dx), (a, xb, c)]
        D = data[b]                           # [C, H, W] f32
        rows = np.zeros((C, nyb * YWIN, W), f32)
        r1 = min(H, rs + nyb * YWIN)
        rows[:, :r1 - rs, :] = D[:, rs:r1, :]
        m4 = rows.reshape(C, nyb, YWIN, NXB, XWIN)
        m4 = np.ascontiguousarray(m4.transpose(2, 4, 1, 3, 0))  # dy,dx,a,xb,c
        m4 = m4.reshape(128, nyb * NXB * C)

        wm = np.zeros((128, WCm), f32)
        for gi, gd in enumerate(gens):
            cb = colbin[(ci, gi)]
            for m in gd["mms"]:
                a, b_, cl, ch, wc = m["a"], m["b"], m["cl"], m["ch"], m["wc"]
                q = cb[cl:ch]
                sel = q >= 0
                if not sel.any():
                    continue
                qs = q[sel]
                jsel = np.nonzero(sel)[0]
                xs = np.arange(XWIN * b_, XWIN * (b_ + 1))
                kxv = kx[qs][:, xs]                      # [n, 32]
                ys = rs + YWIN * a + np.arange(YWIN)
                valid = ys < H
                kyv = np.zeros((len(qs), YWIN), f32)
                kyv[:, valid] = ky[qs][:, ys[valid]]     # [n, 4]
                vals = kyv[:, :, None] * kxv[:, None, :]
                wm[:, wc + jsel] = vals.reshape(len(qs), 128).T
        in_maps.append({"mp": _bf16(m4), "wm": _bf16(wm)})
    return in_maps


# ----------------------------------------------------------------------------
# host emulation (for plan debugging; mirrors the device program exactly)
# ----------------------------------------------------------------------------

def _emulate(plan, in_maps):
    meta = plan["meta"]
    nslots = meta["nslots"]
    outs = []
    for ci in range(NCORES):
        m4 = in_maps[ci]["mp"].astype(f32)
        wm = in_maps[ci]["wm"].astype(f32)
        o = np.zeros((128, nslots), f32)
        for (g, capw, out_off, mms) in meta["gens"]:
            ps = np.zeros((128, GEN_COLS), f32)
            for (a, b_, cl, ch, wc) in mms:
                blk = m4[:, (a * NXB + b_) * C:(a * NXB + b_ + 1) * C]
                ps[:, cl:ch] += blk.T @ wm[:, wc:wc + ch - cl]
            o[:, out_off:out_off + capw] = ps[:, :capw]
        outs.append(o)
    return outs


def _gather(plan, outs):
    meta = plan["meta"]
    N = plan["batch"].shape[0] // (POOLED * POOLED)
    flat = np.zeros((N * POOLED * POOLED, C), f32)
    for ci in range(NCORES):
        o = outs[ci]
        for gi, (g, capw, out_off, mms) in enumerate(meta["gens"]):
            cb = plan["colbin"][(ci, gi)]
            sel = cb >= 0
            if sel.any():
                flat[cb[sel]] = o[:, out_off:out_off + capw][:, sel].T
    flat[~plan["act"]] = 0.0
    out = flat.reshape(N, POOLED, POOLED, C).transpose(0, 3, 1, 2)
    return np.ascontiguousarray(out)


# ----------------------------------------------------------------------------
# device program
# ----------------------------------------------------------------------------

def _split_drains(nc, mybir, bass_rust):
    for f_ in nc.m.functions:
        for blk in f_.blocks:
            newlist = []
            for ins in blk.instructions:
                wts = list(ins.sync_info.on_wait) if ins.sync_info else []
                if len(wts) > 1 and type(ins).__name__ == "InstDrain":
                    for j, wx in enumerate(wts[1:]):
                        nop = mybir.InstNoOp(name=f"splitw_{id(ins)}_{j}", ins=[], outs=[])
                        nop.engine = ins.engine
                        nop.sync_info = bass_rust.SyncInfo(on_wait=[wx], on_update=[])
                        newlist.append(nop)
                    ins.sync_info.on_wait = wts[:1]
                newlist.append(ins)
            blk.instructions = newlist


def _build_program(meta, rep=1):
    import concourse.bacc as bacc
    import concourse.mybir as mybir
    import bass_rust
    from concourse.tile import TileContext

    nyb, nslots = meta["nyb"], meta["nslots"]
    WCm = meta["WCm"]
    gens, wm_chunks = meta["gens"], meta["wm_chunks"]
    dt = mybir.dt.bfloat16

    nc = bacc.Bacc()
    mp = nc.declare_dram_parameter("mp", [128, nyb * NXB * C], dt, isOutput=False)
    wm = nc.declare_dram_parameter("wm", [128, WCm], dt, isOutput=False)
    o = nc.declare_dram_parameter("o", [128, nslots], dt, isOutput=True)

    wm_max = max(hi - lo for (_, _, lo, hi) in wm_chunks) if wm_chunks else 8

    with TileContext(nc) as tc:
        with (
            tc.tile_pool(name="const", bufs=1) as constp,
            tc.tile_pool(name="wp", bufs=3) as wpool,
            tc.tile_pool(name="ps", bufs=2, space="PSUM") as pspool,
        ):
            for _rep in range(rep):
                map_t = constp.tile([128, nyb * NXB * C], dt, tag="map")
                stage = constp.tile([128, nslots], dt, tag="stage")
                # map: 8 slices on SP ring, at yblk boundaries
                nload = min(8, nyb)
                bounds = [int(nyb * i / nload) for i in range(nload + 1)]
                for i in range(nload):
                    a0, a1 = bounds[i], bounds[i + 1]
                    if a1 > a0:
                        nc.sync.dma_start(
                            out=map_t[:, a0 * NXB * C:a1 * NXB * C],
                            in_=mp[:, a0 * NXB * C:a1 * NXB * C])

                wm_t = {}
                for (g0, g1, lo, hi) in wm_chunks:
                    if hi > lo:
                        t = wpool.tile([128, wm_max], dt, tag="wm")
                        nc.scalar.dma_start(out=t[:, :hi - lo], in_=wm[:, lo:hi])
                        for gi in range(g0, g1):
                            wm_t[gi] = (t, lo)

                flush_from = 0
                for gi, (g, capw, out_off, mms) in enumerate(gens):
                    ps = pspool.tile([128, GEN_COLS], mybir.dt.float32, tag="ps")
                    n = len(mms)
                    t, lo = wm_t[gi]
                    for mi, (a, b_, cl, ch, wc) in enumerate(mms):
                        blk = slice((a * NXB + b_) * C, (a * NXB + b_ + 1) * C)
                        nc.tensor.matmul(
                            ps[:, cl:ch], map_t[:, blk],
                            t[:, wc - lo:wc - lo + ch - cl],
                            start=(mi == 0), stop=(mi == n - 1))
                    cw = min(-(-capw // 8) * 8, GEN_COLS)
                    nc.vector.tensor_copy(stage[:, out_off:out_off + cw],
                                          ps[:, :cw])
                    if gi % 2 == 1 or gi == len(gens) - 1:
                        hi = out_off + cw
                        nc.scalar.dma_start(out=o[:, flush_from:hi],
                                            in_=stage[:, flush_from:hi])
                        flush_from = hi

    _split_drains(nc, mybir, bass_rust)
    nc.finalize()
    return nc


_prog_cache = {}


def _get_program(meta, rep=1):
    key = (meta["gens"], meta["nyb"], meta["nslots"], meta["WCm"], rep)
    if key not in _prog_cache:
        _prog_cache[key] = _build_program(meta, rep=rep)
    return _prog_cache[key]


def _run(nc, in_maps):
    from concourse.bass_utils import run_bass_kernel_spmd
    last_err = None
    for _attempt in range(3):
        try:
            res = run_bass_kernel_spmd(nc, in_maps, list(range(NCORES)))
            return res.results
        except Exception as e:  # transient device wedge -> retry
            last_err = e
            time.sleep(2.0)
    raise last_err


# ----------------------------------------------------------------------------
# public entry
# ----------------------------------------------------------------------------

def kernel(data, rois, offset):
    data = np.asarray(data, f32)
    rois = np.asarray(rois, f32)
    offset = np.asarray(offset, f32)
    N = rois.shape[0]

    plan = _plan(rois, offset)
    if len(plan["meta"]["gens"]) == 0:
        return np.zeros((N, C, POOLED, POOLED), f32)
    in_maps = _build_inputs(plan, data)
    nc = _get_program(plan["meta"])
    results = _run(nc, in_maps)
    outs = [results[ci]["o"].astype(f32) for ci in range(NCORES)]
    return _gather(plan, outs)
